# revision 7
# baseline (speedup 1.0000x reference)
"""AILSTM kernel for 8 TRN2 NeuronCores.

Key algebraic identity: the reference's (B, Lq, La, H) interaction
tensor is never materialized, because
    max_i tanh(pq[i] + pa[j] + ba) == tanh(max_i(pq[i]) + pa[j] + ba)
(the broadcast add is constant in i; tanh is monotone).  The problem
collapses to two 512-step LSTM scans (strictly latency-bound) plus
tiny projections.  Since B=4 and the recurrence is sequential, all 8
cores run the identical replicated program (q and a batched as 8 rows)
and the output is read from core 0 — no collectives.

Layouts:
 - token order b-major: column j = b*512 + t, b in [0..7] = [q0..q3, a0..a3]
 - gates in PSUM: one (128, 1024) tile per chunk parity, column
   slot*256 + b*32 + tt with slot order [g, i] (bank 0), [f, o] (bank 1)
 - X = emb @ W_ih matmul'd straight into those PSUM banks one chunk
   ahead (double-buffered), so the recurrent matmul accumulates onto X
   (start=False) and sigmoid/tanh read PSUM with the per-partition
   b_lstm bias folded into the activation.
 - prologue work (embedding gathers, PE transposes, X matmuls) is
   interleaved into the step stream so it fills engine idle slots
   without ever blocking the in-order PE queue on a PSUM WAR hazard:
   X(c+1) is emitted during chunk c (its parity was last read in
   chunk c-1, which already finished).
"""

import sys

import numpy as np

if "/opt/trn_rl_repo" not in sys.path:
    sys.path.insert(0, "/opt/trn_rl_repo")

B, L, E, H, V = 4, 512, 300, 128, 50000
G4 = 4 * H
NB = 2 * B           # 8 combined batch rows
TOK = NB * L         # 4096
TC = 32              # steps per PSUM chunk
NCHUNK = L // TC     # 16
PSUM_SLOT = {"g": 0, "i": 1, "f": 2, "o": 3}
BLSTM_COL = {"i": 0, "f": 1, "g": 2, "o": 3}
GATES = ("g", "i", "f", "o")


def build():
    import concourse.bacc as bacc
    import concourse.bass as bass
    import concourse.mybir as mybir
    import concourse.tile as tile
    from concourse.alu_op_type import AluOpType as OP
    from concourse.masks import make_identity

    F32 = mybir.dt.float32
    I32 = mybir.dt.int32
    AF = mybir.ActivationFunctionType
    AX = mybir.AxisListType

    nc = bacc.Bacc("TRN2", target_bir_lowering=False, debug=False)

    qc = nc.dram_tensor("question_content", (B, L), I32, kind="ExternalInput")
    ac = nc.dram_tensor("answer_content", (B, L), I32, kind="ExternalInput")
    tbl = nc.dram_tensor("embed_table", (V, E), F32, kind="ExternalInput")
    w_ih = nc.dram_tensor("W_ih", (E, G4), F32, kind="ExternalInput")
    w_hh = nc.dram_tensor("W_hh", (H, G4), F32, kind="ExternalInput")
    b_lstm = nc.dram_tensor("b_lstm", (G4,), F32, kind="ExternalInput")
    wa = nc.dram_tensor("Wa", (2 * H, H), F32, kind="ExternalInput")
    ba = nc.dram_tensor("ba", (H,), F32, kind="ExternalInput")
    wq = nc.dram_tensor("Wq", (2 * H, 1), F32, kind="ExternalInput")
    wans = nc.dram_tensor("Wans", (2 * H, 1), F32, kind="ExternalInput")
    wlast = nc.dram_tensor("Wlast", (2 * H, 2), F32, kind="ExternalInput")
    blast = nc.dram_tensor("blast", (2,), F32, kind="ExternalInput")
    # bq / bans are softmax-shift-invariant: intentionally unused.
    nc.dram_tensor("bq", (1,), F32, kind="ExternalInput")
    nc.dram_tensor("bans", (1,), F32, kind="ExternalInput")
    score_out = nc.dram_tensor("score_out", (B, 2), F32, kind="ExternalOutput")
    predict_out = nc.dram_tensor("predict_out", (B, 1), F32, kind="ExternalOutput")

    with tile.TileContext(nc) as tc, \
         tc.tile_pool(name="const", bufs=1) as cst, \
         tc.tile_pool(name="big", bufs=1) as big:
        with (
            tc.tile_pool(name="gath", bufs=8) as gpool,
            tc.tile_pool(name="state", bufs=4) as st,
            tc.tile_pool(name="psx", bufs=1, space="PSUM") as psx,
            tc.tile_pool(name="pst", bufs=2, space="PSUM") as pst,
        ):
            # ---- weights / constants ----
            ident = cst.tile([128, 128], F32)
            make_identity(nc, ident[:])
            whh = cst.tile([H, G4], F32)
            nc.sync.dma_start(whh[:], w_hh[:])
            wih0 = cst.tile([128, G4], F32)
            wih1 = cst.tile([128, G4], F32)
            wih2 = cst.tile([44, G4], F32)
            nc.sync.dma_start(wih0[:], w_ih[0:128, :])
            nc.sync.dma_start(wih1[:], w_ih[128:256, :])
            nc.sync.dma_start(wih2[:], w_ih[256:300, :])
            bl = cst.tile([H, 4], F32)  # columns: i, f, g, o
            nc.sync.dma_start(bl[:], b_lstm[:].rearrange("(g p) -> p g", p=H))
            waq = cst.tile([H, H], F32)
            waa = cst.tile([H, H], F32)
            nc.sync.dma_start(waq[:], wa[0:H, :])
            nc.sync.dma_start(waa[:], wa[H:2 * H, :])
            ba_t = cst.tile([H, 1], F32)
            nc.sync.dma_start(ba_t[:], ba[:].rearrange("(o p) -> p o", p=H))
            wq_t = cst.tile([H, 2], F32)
            nc.sync.dma_start(wq_t[:, 0:1], wq[0:H, :])
            nc.sync.dma_start(wq_t[:, 1:2], wq[H:2 * H, :])
            wans_t = cst.tile([H, 2], F32)
            nc.sync.dma_start(wans_t[:, 0:1], wans[0:H, :])
            nc.sync.dma_start(wans_t[:, 1:2], wans[H:2 * H, :])
            wl_t = cst.tile([H, 4], F32)  # free = (k, class)
            nc.sync.dma_start(wl_t[:, 0:2], wlast[0:H, :])
            nc.sync.dma_start(wl_t[:, 2:4], wlast[H:2 * H, :])
            blast_t = cst.tile([1, 2], F32)
            nc.sync.dma_start(blast_t[:], blast[:].rearrange("(o c) -> o c", o=1))
            ones_row = cst.tile([1, 128], F32)
            nc.gpsimd.memset(ones_row[:], 1.0)

            # ---- token index tiles: (t-part, b-col) ----
            cont = cst.tile([B, 2 * L], I32)
            nc.sync.dma_start(cont[:, 0:L], qc[:])
            nc.sync.dma_start(cont[:, L:2 * L], ac[:])
            contf = cst.tile([128, 2 * L], F32)
            nc.gpsimd.memset(contf[:], 0.0)
            nc.vector.tensor_copy(contf[0:B, :], cont[:])

            idx_i = []
            for k in range(4):
                idf = cst.tile([128, NB], F32, tag=f"idxf{k}")
                tp = pst.tile([128, 128], F32, tag="tposer")
                nc.tensor.transpose(tp[:], contf[:, k * 128:(k + 1) * 128], ident[:])
                nc.vector.tensor_copy(idf[:, 0:B], tp[:, 0:B])
                tp2 = pst.tile([128, 128], F32, tag="tposer")
                nc.tensor.transpose(tp2[:], contf[:, L + k * 128:L + (k + 1) * 128], ident[:])
                nc.vector.tensor_copy(idf[:, B:NB], tp2[:, 0:B])
                ii = cst.tile([128, NB], I32, tag=f"idxi{k}")
                nc.vector.tensor_copy(ii[:], idf[:])
                idx_i.append(ii)

            # ---- big SBUF tensors ----
            e0 = big.tile([128, TOK], F32)
            e1 = big.tile([128, TOK], F32)
            e2 = big.tile([44, TOK], F32)
            h_all = big.tile([H, TOK], F32)
            ha = h_all[:].rearrange("p (b t) -> p b t", b=NB)

            grows = {}  # (k, b) -> gather tile

            def emit_gather(k, b):
                g = gpool.tile([128, E], F32, tag="grow")
                nc.gpsimd.indirect_dma_start(
                    out=g[:], out_offset=None, in_=tbl[:],
                    in_offset=bass.IndirectOffsetOnAxis(ap=idx_i[k][:, b:b + 1], axis=0),
                )
                grows[(k, b)] = g

            def emit_tpose(k, b, part):
                g = grows[(k, b)]
                lo, hi = ((0, 128), (128, 256), (256, 300))[part]
                w = hi - lo
                tp = pst.tile([128, 128], F32, tag="tposer")
                nc.tensor.transpose(tp[0:w, :], g[:, lo:hi], ident[:])
                dst = (e0, e1, e2)[part]
                col0 = b * L + k * 128
                nc.vector.tensor_copy(dst[0:w, col0:col0 + 128], tp[0:w, :])

            xp = [psx.tile([128, 1024], F32, tag=f"xp{p}", name=f"xp{p}") for p in range(2)]

            def emit_x_mm(chunk, gname, kt):
                slot = PSUM_SLOT[gname]
                gcol = BLSTM_COL[gname]
                wtile, kk = ((wih0, 128), (wih1, 128), (wih2, 44))[kt]
                src = (e0, e1, e2)[kt]
                rhs = src[0:kk, :].rearrange("p (b t) -> p b t", b=NB)[:, :, chunk * TC:(chunk + 1) * TC]
                out = xp[chunk % 2][:].rearrange("p (s b t) -> p s b t", s=4, b=NB)[:, slot, :, :]
                nc.tensor.matmul(
                    out=out, lhsT=wtile[:, gcol * H:(gcol + 1) * H], rhs=rhs,
                    start=(kt == 0), stop=(kt == 2), skip_group_check=True,
                )

            # ---- prologue: k-block 0 fully, X for chunks 0 and 1 ----
            for b in range(NB):
                emit_gather(0, b)
            for b in range(NB):
                for part in range(3):
                    emit_tpose(0, b, part)
            for g in GATES:
                for kt in range(3):
                    emit_x_mm(0, g, kt)
            for g in GATES:
                for kt in range(3):
                    emit_x_mm(1, g, kt)

            # ---- deferred work schedule: step -> [thunks] ----
            sched = {}

            def at(t, fn):
                sched.setdefault(t, []).append(fn)

            # gathers for k-block k during chunk 4*(k-1)
            for k in (1, 2, 3):
                c0 = 4 * (k - 1) * TC
                for b in range(NB):
                    at(c0 + 1 + 3 * b, lambda k=k, b=b: emit_gather(k, b))
            # transposes for k-block k during chunk 4k-2
            for k in (1, 2, 3):
                c0 = (4 * k - 2) * TC
                i = 0
                for b in range(NB):
                    for part in range(3):
                        at(c0 + 2 + i, lambda k=k, b=b, part=part: emit_tpose(k, b, part))
                        i += 1
            # X matmuls for chunk c+1 during chunk c (parity safe: last
            # reader of that parity was chunk c-1, already done)
            for c in range(1, NCHUNK - 1):
                i = 0
                for g in GATES:
                    for kt in range(3):
                        at(c * TC + 2 + 2 * i, lambda c=c, g=g, kt=kt: emit_x_mm(c + 1, g, kt))
                        i += 1

            # ---- the 512-step LSTM recurrence ----
            c_prev = None
            for t in range(L):
                par = (t // TC) % 2
                tt = t % TC
                xv = xp[par][:].rearrange("p (s b t) -> p s b t", s=4, b=NB)
                gate_in = {g: xv[:, PSUM_SLOT[g], :, tt] for g in GATES}

                if t > 0:
                    h_prev = ha[:, :, t - 1]
                    for gname in GATES:
                        gcol = BLSTM_COL[gname]
                        nc.tensor.matmul(
                            out=gate_in[gname],
                            lhsT=whh[:, gcol * H:(gcol + 1) * H],
                            rhs=h_prev,
                            start=False, stop=True, skip_group_check=True,
                        )

                g_s = st.tile([H, NB], F32, tag="g_s")
                i_s = st.tile([H, NB], F32, tag="i_s")
                f_s = st.tile([H, NB], F32, tag="f_s")
                o_s = st.tile([H, NB], F32, tag="o_s")
                nc.scalar.activation(g_s[:], gate_in["g"], AF.Tanh, bias=bl[:, 2:3])
                nc.scalar.activation(i_s[:], gate_in["i"], AF.Sigmoid, bias=bl[:, 0:1])
                nc.scalar.activation(f_s[:], gate_in["f"], AF.Sigmoid, bias=bl[:, 1:2])
                nc.scalar.activation(o_s[:], gate_in["o"], AF.Sigmoid, bias=bl[:, 3:4])

                c_new = st.tile([H, NB], F32, tag="c")
                if t == 0:
                    nc.vector.tensor_tensor(c_new[:], i_s[:], g_s[:], op=OP.mult)
                else:
                    ig = st.tile([H, NB], F32, tag="ig")
                    fc = st.tile([H, NB], F32, tag="fc")
                    nc.vector.tensor_tensor(ig[:], i_s[:], g_s[:], op=OP.mult)
                    nc.vector.tensor_tensor(fc[:], f_s[:], c_prev[:], op=OP.mult)
                    nc.vector.tensor_tensor(c_new[:], ig[:], fc[:], op=OP.add)
                th = st.tile([H, NB], F32, tag="th")
                nc.scalar.activation(th[:], c_new[:], AF.Tanh)
                nc.vector.tensor_tensor(ha[:, :, t], o_s[:], th[:], op=OP.mult)
                c_prev = c_new

                for fn in sched.get(t, ()):
                    fn()

        # ---- phase 2: projections, maxes via monotonicity, attention ----
        with (
            tc.tile_pool(name="p2", bufs=1, space="PSUM") as p2,
            tc.tile_pool(name="s2", bufs=2) as s2,
            tc.tile_pool(name="sm", bufs=4) as sm,
        ):
            featq = s2.tile([H, B], F32, tag="featq")
            feata = s2.tile([H, B], F32, tag="feata")

            for b in range(B):
                pq = p2.tile([H, L], F32, tag="pq")
                pa = p2.tile([H, L], F32, tag="pa")
                nc.tensor.matmul(out=pq[:], lhsT=waq[:], rhs=h_all[:, b * L:(b + 1) * L],
                                 start=True, stop=True)
                nc.tensor.matmul(out=pa[:], lhsT=waa[:], rhs=h_all[:, (B + b) * L:(B + b + 1) * L],
                                 start=True, stop=True)
                mq = sm.tile([H, 1], F32, tag="mq")
                ma = sm.tile([H, 1], F32, tag="ma")
                nc.vector.tensor_reduce(mq[:], pq[:], axis=AX.X, op=OP.max)
                nc.vector.tensor_reduce(ma[:], pa[:], axis=AX.X, op=OP.max)
                bq_bias = sm.tile([H, 1], F32, tag="bqb")
                bab_bias = sm.tile([H, 1], F32, tag="bab")
                nc.vector.tensor_tensor(bq_bias[:], mq[:], ba_t[:], op=OP.add)
                nc.vector.tensor_tensor(bab_bias[:], ma[:], ba_t[:], op=OP.add)
                r_q = s2.tile([H, L], F32, tag="r_q")
                r_a = s2.tile([H, L], F32, tag="r_a")
                # r_q[j] = tanh(mq + proj_a[j] + ba); r_a[i] = tanh(proj_q[i] + ma + ba)
                nc.scalar.activation(r_q[:], pa[:], AF.Tanh, bias=bq_bias[:])
                nc.scalar.activation(r_a[:], pq[:], AF.Tanh, bias=bab_bias[:])

                for (hslice, r, wvec, feat) in (
                    (h_all[:, b * L:(b + 1) * L], r_q, wq_t, featq),
                    (h_all[:, (B + b) * L:(B + b + 1) * L], r_a, wans_t, feata),
                ):
                    lg = p2.tile([1, L], F32, tag="lg")
                    nc.tensor.matmul(out=lg[:], lhsT=wvec[:, 0:1], rhs=hslice,
                                     start=True, stop=False)
                    nc.tensor.matmul(out=lg[:], lhsT=wvec[:, 1:2], rhs=r[:],
                                     start=False, stop=True)
                    nmax = sm.tile([1, 1], F32, tag="nmax")
                    nc.vector.tensor_reduce(nmax[:], lg[:], axis=AX.X, op=OP.max, negate=True)
                    ex = sm.tile([1, L], F32, tag="ex")
                    sume = sm.tile([1, 1], F32, tag="sume")
                    nc.scalar.activation(ex[:], lg[:], AF.Exp, bias=nmax[:], accum_out=sume[:])
                    rec = sm.tile([1, 1], F32, tag="rec")
                    nc.vector.reciprocal(rec[:], sume[:])
                    alpha = sm.tile([1, L], F32, tag="alpha")
                    nc.vector.tensor_scalar(alpha[:], ex[:], rec[:], None, op0=OP.mult)
                    ab = p2.tile([H, L], F32, tag="ab")
                    nc.tensor.matmul(out=ab[:], lhsT=ones_row[:], rhs=alpha[:],
                                     start=True, stop=True)
                    wr = s2.tile([H, L], F32, tag="wr")
                    nc.vector.tensor_tensor(wr[:], r[:], ab[:], op=OP.mult)
                    nc.vector.tensor_reduce(feat[:, b:b + 1], wr[:], axis=AX.X, op=OP.add)

            lg2 = p2.tile([B, 2], F32, tag="lg2")
            nc.tensor.matmul(out=lg2[:], lhsT=featq[:], rhs=wl_t[:, 0:2], start=True, stop=False)
            nc.tensor.matmul(out=lg2[:], lhsT=feata[:], rhs=wl_t[:, 2:4], start=False, stop=False)
            nc.tensor.matmul(out=lg2[:], lhsT=ones_row[:, 0:B], rhs=blast_t[:], start=False, stop=True)

            nmax2 = sm.tile([B, 1], F32, tag="nmax2")
            nc.vector.tensor_reduce(nmax2[:], lg2[:], axis=AX.X, op=OP.max, negate=True)
            ex2 = sm.tile([B, 2], F32, tag="ex2")
            se2 = sm.tile([B, 1], F32, tag="se2")
            nc.scalar.activation(ex2[:], lg2[:], AF.Exp, bias=nmax2[:], accum_out=se2[:])
            lns = sm.tile([B, 1], F32, tag="lns")
            nc.scalar.activation(lns[:], se2[:], AF.Ln)
            nlns = sm.tile([B, 1], F32, tag="nlns")
            nc.vector.tensor_scalar(nlns[:], lns[:], -1.0, None, op0=OP.mult)
            sc1 = sm.tile([B, 2], F32, tag="sc1")
            nc.vector.tensor_scalar(sc1[:], lg2[:], nmax2[:], None, op0=OP.add)
            score = sm.tile([B, 2], F32, tag="score")
            nc.vector.tensor_scalar(score[:], sc1[:], nlns[:], None, op0=OP.add)
            pred = sm.tile([B, 1], F32, tag="pred")
            nc.vector.tensor_tensor(pred[:], score[:, 1:2], score[:, 0:1], op=OP.is_gt)

            nc.sync.dma_start(score_out[:], score[:])
            nc.sync.dma_start(predict_out[:], pred[:])

    nc.compile()
    return nc


_NC_CACHE = None
LAST_RESULT = None


def _ensure_axon_hooks_importable():
    """bass_utils imports antenv.axon_hooks unconditionally when tracing;
    some images lack the module.  Provide a no-op fallback so tracing
    degrades instead of crashing (test harness installs the real hook)."""
    if "antenv.axon_hooks" in sys.modules:
        return
    try:
        import antenv.axon_hooks  # noqa: F401
        return
    except ImportError:
        pass
    import types

    mod = types.ModuleType("antenv.axon_hooks")
    state = {"hook": None}
    mod.set_axon_ntff_profile_hook = lambda h: state.__setitem__("hook", h)
    mod.get_axon_ntff_profile_hook = lambda: state["hook"]
    sys.modules["antenv.axon_hooks"] = mod


def kernel(**inputs):
    global _NC_CACHE, LAST_RESULT
    _ensure_axon_hooks_importable()
    from concourse.bass_utils import run_bass_kernel_spmd

    if _NC_CACHE is None:
        _NC_CACHE = build()
    nc = _NC_CACHE

    in_map = {k: np.ascontiguousarray(np.asarray(v)) for k, v in inputs.items()}
    in_maps = [dict(in_map) for _ in range(8)]
    res = run_bass_kernel_spmd(nc, in_maps, core_ids=list(range(8)))
    LAST_RESULT = res
    r0 = res.results[0]
    score = np.asarray(r0["score_out"], dtype=np.float32).reshape(B, 2)
    predict = np.asarray(r0["predict_out"]).reshape(B).astype(np.int32)
    return score, predict


if __name__ == "__main__":
    build()
    print("build ok")
